# revision 7
# baseline (speedup 1.0000x reference)
"""Trainium2 Bass kernel for the spiral-conv architecture block.

Math: the reference's FFT-based "spiral conv" is a first-order complex linear
recurrence per channel d:
    E[t] = phz_d * E[t-1] + phi_d * xn[t],   E[-1] = lci_d
    conv[t] = Re(E[t])
computed as a two-level scan:
  level 1: scaled cumsum within 16-step blocks via one block-diagonal
           lower-triangular matmul (u[j] = phz^-(j%16) * g * phi * xn[j],
           S = BDT @ u, in-block state = phz^(j%16) * S)
  level 2: native tensor_tensor_scan over the 128 per-block sums in
           channel-major layout: derotate by the unit phasor e^(-16i*theta*b)
           so the scan multiplier is the real decay e^(-16a), run two real
           scans (re/im), rerotate.
  cross-core: each core owns half a sample's time axis; the half-boundary
           state (8KB) moves between core pairs with one AllGather, then
           enters as an additive per-block correction t16[b] * init.
Everything else (LayerNorm, fc gate GEMM + SiLU, FFN) is row-local; rows are
sharded 2048 per core (core 2b+h gets sample b, half h). GEMMs run in bf16
with f32 PSUM accumulation; the scan path runs in f32.
"""

import numpy as np
import ml_dtypes

import concourse.bass as bass
from concourse import bacc, mybir
from concourse.bass_utils import run_bass_kernel_spmd
from concourse.tile import TileContext
from concourse.masks import make_identity

B, L, D, FF = 4, 4096, 1024, 2048
NCORES = 8
R = L // 2            # rows per core
TB = 16               # level-1 block length
NBLK = R // TB        # 128 blocks per core
NT = R // 128         # 16 row tiles per core
NDT = D // 128        # 8 channel tiles
EPS = 1e-5
F32 = mybir.dt.float32
BF16 = mybir.dt.bfloat16
AO = mybir.AluOpType
AF = mybir.ActivationFunctionType
BF = ml_dtypes.bfloat16

_CACHE = {}


def build_nc():
    nc = bacc.Bacc("TRN2", target_bir_lowering=False, debug=False,
                   num_devices=NCORES)

    def par(name, shape, dt=F32, out=False):
        return nc.declare_dram_parameter(name, list(shape), dt, isOutput=out)

    x_p = par("x", [R, D])
    out_p = par("out", [R, D], out=True)
    fcw_p = par("fcw", [8, 128, D], BF16)
    w1_p = par("w1", [8, 128, FF], BF16)
    w2_p = par("w2", [16, 128, D], BF16)
    p2_re_p = par("p2_re", [128, D]); p2_im_p = par("p2_im", [128, D])
    a_re_p = par("a_re", [128, D]); a_im_p = par("a_im", [128, D])
    t16_re_p = par("t16_re", [128, D]); t16_im_p = par("t16_im", [128, D])
    bdt_p = par("bdt", [128, 128])
    e8_p = par("e8", [128, 16 * 128])
    ws_p = par("ws", [128, 16 * 128])
    rotc_p = par("rotc", [128, NDT, NBLK]); rots_p = par("rots", [128, NDT, NBLK])
    r16_p = par("r16", [128, NDT])
    p16_re_p = par("p16_re", [128, NDT]); p16_im_p = par("p16_im", [128, NDT])
    p15_re_p = par("p15_re", [128, NDT]); p15_im_p = par("p15_im", [128, NDT])
    xb_re_p = par("xb_re", [128, NDT]); xb_im_p = par("xb_im", [128, NDT])
    sel8_p = par("sel8", [8, 1])
    lcie_re_p = par("lcie_re", [1, D]); lcie_im_p = par("lcie_im", [1, D])
    fcb_p = par("fcb", [1, D], BF16)
    b1_p = par("b1r", [1, FF], BF16)
    b2_p = par("b2r", [1, D], BF16)

    bounce = nc.dram_tensor("bounce", [1, 2 * D], F32)
    gath = nc.dram_tensor("gath", [NCORES, 2 * D], F32, addr_space="Shared")

    with TileContext(nc) as tc:
        with tc.tile_pool(name="singles", bufs=1) as sg, \
             tc.tile_pool(name="wx", bufs=2) as wx, \
             tc.tile_pool(name="wn", bufs=2) as wn, \
             tc.tile_pool(name="wu", bufs=2) as wu, \
             tc.tile_pool(name="wst", bufs=4) as wst, \
             tc.tile_pool(name="ppz", bufs=1, space="PSUM") as ppz, \
             tc.tile_pool(name="ppt", bufs=2, space="PSUM") as ppt, \
             tc.tile_pool(name="ppg", bufs=2, space="PSUM") as ppg:

            # ---- load constants ----
            def load(pool, p, tag=None):
                t = pool.tile(list(p.shape), p.dtype, tag=tag or p.name)
                nc.sync.dma_start(out=t, in_=p[:])
                return t

            p2_re = load(sg, p2_re_p); p2_im = load(sg, p2_im_p)
            bdt = load(sg, bdt_p); e8 = load(sg, e8_p)
            r16 = load(sg, r16_p)
            p16_re = load(sg, p16_re_p); p16_im = load(sg, p16_im_p)
            p15_re = load(sg, p15_re_p); p15_im = load(sg, p15_im_p)
            xb_re = load(sg, xb_re_p); xb_im = load(sg, xb_im_p)
            sel8 = load(sg, sel8_p)
            lcie_re = load(sg, lcie_re_p); lcie_im = load(sg, lcie_im_p)
            fcb = load(sg, fcb_p); b1r = load(sg, b1_p); b2r = load(sg, b2_p)

            fcw = sg.tile([128, 8, D], BF16, tag="fcw")
            nc.sync.dma_start(out=fcw, in_=fcw_p.rearrange("c p d -> p c d"))
            w1 = sg.tile([128, 8, FF], BF16, tag="w1")
            nc.sync.dma_start(out=w1, in_=w1_p.rearrange("c p d -> p c d"))

            id_bf = sg.tile([128, 128], BF16, tag="id_bf")
            make_identity(nc, id_bf)
            id_f32 = sg.tile([128, 128], F32, tag="id_f32")
            make_identity(nc, id_f32)
            ones_1x128 = sg.tile([1, 128], F32, tag="ones_1x128")
            nc.vector.memset(ones_1x128, 1.0)
            ones_1x1 = sg.tile([1, 1], F32, tag="ones_1x1")
            nc.vector.memset(ones_1x1, 1.0)
            ones_bf = sg.tile([1, 128], BF16, tag="ones_bf")
            nc.vector.memset(ones_bf, 1.0)
            eps_t = sg.tile([128, 1], F32, tag="eps")
            nc.vector.memset(eps_t, EPS)

            cq_re = sg.tile([128, D], F32, tag="cq_re")
            cq_im = sg.tile([128, D], F32, tag="cq_im")

            # ---------- helpers ----------
            def layer_norm(xt, dst):
                stats = wst.tile([128, 2, 6], F32, tag="stats")
                nc.vector.bn_stats(out=stats[:, 0, :], in_=xt[:, 0:512])
                nc.vector.bn_stats(out=stats[:, 1, :], in_=xt[:, 512:1024])
                mv = wst.tile([128, 2], F32, tag="mv")
                nc.vector.bn_aggr(out=mv, in_=stats)
                rstd = wst.tile([128, 1], F32, tag="rstd")
                nc.scalar.activation(out=rstd, in_=mv[:, 1:2], func=AF.Sqrt,
                                     bias=eps_t, scale=1.0)
                nc.vector.reciprocal(out=rstd, in_=rstd)
                negm = wst.tile([128, 1], F32, tag="negm")
                nc.vector.tensor_scalar_mul(out=negm, in0=mv[:, 0:1], scalar1=-1.0)
                nc.vector.tensor_scalar(out=dst, in0=xt, scalar1=negm,
                                        scalar2=rstd, op0=AO.add, op1=AO.mult)

            def make_u(xn):
                # u_re fresh; u_im overwrites xn in place
                u_re = wu.tile([128, D], F32, tag="u_re")
                nc.vector.tensor_mul(out=u_re, in0=xn, in1=p2_re)
                nc.vector.tensor_mul(out=xn, in0=xn, in1=p2_im)
                return u_re, xn

            # ============ PHASES A+B (scoped pool, freed before C) ========
            with tc.tile_pool(name="convB", bufs=1) as cvb:
                be_re = cvb.tile([128, D], F32, tag="be_re")
                be_im = cvb.tile([128, D], F32, tag="be_im")
                ws = load(cvb, ws_p)
                t16_re = load(cvb, t16_re_p); t16_im = load(cvb, t16_im_p)

                # -------- phase A: block sums of u, accumulated in PSUM ----
                bp_re = ppz.tile([128, D], F32, tag="z_re")
                bp_im = ppz.tile([128, D], F32, tag="z_im")
                for i in range(NT):
                    xt = wx.tile([128, D], F32, tag="xt")
                    nc.sync.dma_start(out=xt, in_=x_p[i * 128:(i + 1) * 128, :])
                    xn = wn.tile([128, D], F32, tag="xn")
                    layer_norm(xt, xn)
                    u_re, u_im = make_u(xn)
                    wsl = ws[:, i * 128:(i + 1) * 128]
                    for ch in range(2):
                        c = slice(ch * 512, (ch + 1) * 512)
                        nc.tensor.matmul(bp_re[:, c], lhsT=wsl, rhs=u_re[:, c],
                                         start=(i == 0), stop=(i == NT - 1))
                        nc.tensor.matmul(bp_im[:, c], lhsT=wsl, rhs=u_im[:, c],
                                         start=(i == 0), stop=(i == NT - 1))
                nc.vector.tensor_copy(out=be_re, in_=bp_re)
                nc.vector.tensor_copy(out=be_im, in_=bp_im)

                # -------- phase B: level-2 scan + cross-core exchange ------
                rotc = load(cvb, rotc_p); rots = load(cvb, rots_p)
                cb_re = cvb.tile([128, NDT, 128], F32, tag="cb_re")
                cb_im = cvb.tile([128, NDT, 128], F32, tag="cb_im")
                cb2_re = cvb.tile([128, NDT, 128], F32, tag="cb2_re")
                cb2_im = cvb.tile([128, NDT, 128], F32, tag="cb2_im")
                # transpose block-major -> channel-major
                for dt in range(NDT):
                    cs_ = slice(dt * 128, (dt + 1) * 128)
                    tp = ppt.tile([128, 128], F32, tag="tp")
                    nc.tensor.transpose(tp, be_re[:, cs_], id_f32)
                    nc.vector.tensor_copy(out=cb_re[:, dt, :], in_=tp)
                    tp2 = ppt.tile([128, 128], F32, tag="tp")
                    nc.tensor.transpose(tp2, be_im[:, cs_], id_f32)
                    nc.vector.tensor_copy(out=cb_im[:, dt, :], in_=tp2)
                # derotate: beta = be * (rotc - i*rots)  -> cb2
                tmpb = cvb.tile([128, NDT, 128], F32, tag="tmpb")
                nc.vector.tensor_tensor(out=cb2_re, in0=cb_re, in1=rotc, op=AO.mult)
                nc.vector.tensor_tensor(out=tmpb, in0=cb_im, in1=rots, op=AO.mult)
                nc.vector.tensor_add(out=cb2_re, in0=cb2_re, in1=tmpb)
                nc.vector.tensor_tensor(out=cb2_im, in0=cb_im, in1=rotc, op=AO.mult)
                nc.vector.tensor_tensor(out=tmpb, in0=cb_re, in1=rots, op=AO.mult)
                nc.vector.tensor_sub(out=cb2_im, in0=cb2_im, in1=tmpb)
                # real scans: w[b] = r16*w[b-1] + beta[b]  -> cb
                for dt in range(NDT):
                    rb = r16[:, dt:dt + 1].to_broadcast([128, 128])
                    nc.vector.tensor_tensor_scan(
                        out=cb_re[:, dt, :], data0=rb, data1=cb2_re[:, dt, :],
                        initial=0.0, op0=AO.mult, op1=AO.add)
                    nc.vector.tensor_tensor_scan(
                        out=cb_im[:, dt, :], data0=rb, data1=cb2_im[:, dt, :],
                        initial=0.0, op0=AO.mult, op1=AO.add)
                # rerotate: C = w * (rotc + i*rots) -> cb2
                nc.vector.tensor_tensor(out=cb2_re, in0=cb_re, in1=rotc, op=AO.mult)
                nc.vector.tensor_tensor(out=tmpb, in0=cb_im, in1=rots, op=AO.mult)
                nc.vector.tensor_sub(out=cb2_re, in0=cb2_re, in1=tmpb)
                nc.vector.tensor_tensor(out=cb2_im, in0=cb_im, in1=rotc, op=AO.mult)
                nc.vector.tensor_tensor(out=tmpb, in0=cb_re, in1=rots, op=AO.mult)
                nc.vector.tensor_add(out=cb2_im, in0=cb2_im, in1=tmpb)
                C_re, C_im = cb2_re, cb2_im
                W_re, W_im = cb_re, cb_im      # scratch from here on

                # X_end = p15*C[:,:,127] + xbase  (channel-major [128, NDT])
                xe_re = cvb.tile([128, NDT], F32, tag="xe_re")
                xe_im = cvb.tile([128, NDT], F32, tag="xe_im")
                tt1 = cvb.tile([128, NDT], F32, tag="tt1")
                nc.vector.tensor_mul(out=xe_re, in0=C_re[:, :, 127], in1=p15_re)
                nc.vector.tensor_mul(out=tt1, in0=C_im[:, :, 127], in1=p15_im)
                nc.vector.tensor_sub(out=xe_re, in0=xe_re, in1=tt1)
                nc.vector.tensor_add(out=xe_re, in0=xe_re, in1=xb_re)
                nc.vector.tensor_mul(out=xe_im, in0=C_re[:, :, 127], in1=p15_im)
                nc.vector.tensor_mul(out=tt1, in0=C_im[:, :, 127], in1=p15_re)
                nc.vector.tensor_add(out=xe_im, in0=xe_im, in1=tt1)
                nc.vector.tensor_add(out=xe_im, in0=xe_im, in1=xb_im)
                nc.sync.dma_start(
                    out=bounce[0:1, 0:D].rearrange("o (c p) -> (o p) c", p=128),
                    in_=xe_re)
                nc.sync.dma_start(
                    out=bounce[0:1, D:2 * D].rearrange("o (c p) -> (o p) c", p=128),
                    in_=xe_im)
                nc.gpsimd.collective_compute(
                    "AllGather", AO.bypass,
                    replica_groups=[list(range(NCORES))],
                    ins=[bounce[:, :]], outs=[gath[:, :]])
                gath_sb = cvb.tile([NCORES, 2 * D], F32, tag="gath_sb")
                nc.sync.dma_start(out=gath_sb, in_=gath[:, :])

                # CS = p16 * C (complex) -> W
                p16b_re = p16_re[:, :, None].to_broadcast([128, NDT, 128])
                p16b_im = p16_im[:, :, None].to_broadcast([128, NDT, 128])
                nc.vector.tensor_tensor(out=W_re, in0=C_re, in1=p16b_re, op=AO.mult)
                nc.vector.tensor_tensor(out=tmpb, in0=C_im, in1=p16b_im, op=AO.mult)
                nc.vector.tensor_sub(out=W_re, in0=W_re, in1=tmpb)
                nc.vector.tensor_tensor(out=W_im, in0=C_im, in1=p16b_re, op=AO.mult)
                nc.vector.tensor_tensor(out=tmpb, in0=C_re, in1=p16b_im, op=AO.mult)
                nc.vector.tensor_add(out=W_im, in0=W_im, in1=tmpb)

                # transpose back to block-major into be (dead), then shift
                for dt in range(NDT):
                    cs_ = slice(dt * 128, (dt + 1) * 128)
                    tp = ppt.tile([128, 128], F32, tag="tp")
                    nc.tensor.transpose(tp, W_re[:, dt, :], id_f32)
                    nc.vector.tensor_copy(out=be_re[:, cs_], in_=tp)
                    tp2 = ppt.tile([128, 128], F32, tag="tp")
                    nc.tensor.transpose(tp2, W_im[:, dt, :], id_f32)
                    nc.vector.tensor_copy(out=be_im[:, cs_], in_=tp2)
                cq1_re = C_re.rearrange("p a b -> p (a b)")
                cq1_im = C_im.rearrange("p a b -> p (a b)")
                nc.vector.memset(cq1_re[0:1, :], 0.0)
                nc.vector.memset(cq1_im[0:1, :], 0.0)
                nc.sync.dma_start(out=cq1_re[1:128, :], in_=be_re[0:127, :])
                nc.sync.dma_start(out=cq1_im[1:128, :], in_=be_im[0:127, :])

                # init = sel8 @ gath + lci_eff -> [1, 2D], then broadcast
                init_sb = cvb.tile([1, 2 * D], F32, tag="init_sb")
                for ch in range(4):
                    c = slice(ch * 512, (ch + 1) * 512)
                    lc = (lcie_re, lcie_im)[ch // 2][
                        :, (ch % 2) * 512:(ch % 2) * 512 + 512]
                    ps = ppg.tile([1, 512], F32, tag="gemm")
                    nc.tensor.matmul(ps, lhsT=sel8, rhs=gath_sb[:, c],
                                     start=True, stop=False)
                    nc.tensor.matmul(ps, lhsT=ones_1x1, rhs=lc,
                                     start=False, stop=True)
                    nc.vector.tensor_copy(out=init_sb[:, c], in_=ps)
                ib_re = ppz.tile([128, D], F32, tag="z_re")
                ib_im = ppz.tile([128, D], F32, tag="z_im")
                for ch in range(2):
                    c = slice(ch * 512, (ch + 1) * 512)
                    nc.tensor.matmul(ib_re[:, c], lhsT=ones_1x128,
                                     rhs=init_sb[:, ch * 512:(ch + 1) * 512],
                                     start=True, stop=True)
                    nc.tensor.matmul(ib_im[:, c], lhsT=ones_1x128,
                                     rhs=init_sb[:, D + ch * 512:D + (ch + 1) * 512],
                                     start=True, stop=True)

                # CQ = cq1 + t16 * init  (complex)
                tmq_re = W_re.rearrange("p a b -> p (a b)")
                tmq_im = W_im.rearrange("p a b -> p (a b)")
                nc.vector.tensor_mul(out=tmq_re, in0=t16_re, in1=ib_re)
                nc.vector.tensor_mul(out=tmq_im, in0=t16_im, in1=ib_im)
                nc.vector.tensor_sub(out=tmq_re, in0=tmq_re, in1=tmq_im)
                nc.vector.tensor_add(out=cq_re, in0=cq1_re, in1=tmq_re)
                nc.vector.tensor_mul(out=tmq_re, in0=t16_re, in1=ib_im)
                nc.vector.tensor_mul(out=tmq_im, in0=t16_im, in1=ib_re)
                nc.vector.tensor_add(out=tmq_re, in0=tmq_re, in1=tmq_im)
                nc.vector.tensor_add(out=cq_im, in0=cq1_im, in1=tmq_re)

            # ================= PHASE C: main pass =================
            wc_cm = tc.tile_pool(name="wC", bufs=1)
            wc = wc_cm.__enter__()
            wv = wc
            wbf = wc
            w2 = wc.tile([128, 16, D], BF16, tag="w2")
            nc.sync.dma_start(out=w2, in_=w2_p.rearrange("c p d -> p c d"))
            a_re = load(wc, a_re_p); a_im = load(wc, a_im_p)

            def transpose_bf(src_ap, nchunk, tag):
                dstT = wbf.tile([128, nchunk, 128], BF16, tag=tag)
                for kc in range(nchunk):
                    tp = ppt.tile([128, 128], BF16, tag="tp")
                    nc.tensor.transpose(tp, src_ap[:, kc * 128:(kc + 1) * 128],
                                        id_bf)
                    nc.vector.tensor_copy(out=dstT[:, kc, :], in_=tp)
                return dstT

            for i in range(NT):
                xt = wx.tile([128, D], F32, tag="xt")
                nc.sync.dma_start(out=xt, in_=x_p[i * 128:(i + 1) * 128, :])
                xn = wn.tile([128, D], F32, tag="xn")
                layer_norm(xt, xn)
                u_re, u_im = make_u(xn)
                z_re = ppz.tile([128, D], F32, tag="z_re")
                z_im = ppz.tile([128, D], F32, tag="z_im")
                for ch in range(2):
                    c = slice(ch * 512, (ch + 1) * 512)
                    nc.tensor.matmul(z_re[:, c], lhsT=bdt, rhs=u_re[:, c],
                                     start=True, stop=False)
                    nc.tensor.matmul(z_re[:, c],
                                     lhsT=e8[:, i * 128:(i + 1) * 128],
                                     rhs=cq_re[:, c],
                                     start=False, stop=True)
                    nc.tensor.matmul(z_im[:, c], lhsT=bdt, rhs=u_im[:, c],
                                     start=True, stop=False)
                    nc.tensor.matmul(z_im[:, c],
                                     lhsT=e8[:, i * 128:(i + 1) * 128],
                                     rhs=cq_im[:, c],
                                     start=False, stop=True)
                # conv = a_re*z_re - a_im*z_im  -> cv1
                cv1 = wv.tile([128, D], F32, tag="cv1")
                cv2 = wv.tile([128, D], F32, tag="cv2")
                nc.vector.tensor_mul(out=cv1, in0=a_re, in1=z_re)
                nc.vector.tensor_mul(out=cv2, in0=a_im, in1=z_im)
                nc.vector.tensor_sub(out=cv1, in0=cv1, in1=cv2)
                # fc gate: y = silu(x @ fcw + fcb)
                x_bf = wbf.tile([128, D], BF16, tag="x_bf")
                nc.gpsimd.tensor_copy(out=x_bf, in_=xt)
                xT = transpose_bf(x_bf, 8, "xT")
                y_bf = wbf.tile([128, D], BF16, tag="y_bf")
                for nf in range(2):
                    c = slice(nf * 512, (nf + 1) * 512)
                    yp = ppg.tile([128, 512], F32, tag="gemm")
                    nc.tensor.matmul(yp, lhsT=ones_bf, rhs=fcb[:, c],
                                     start=True, stop=False)
                    for kc in range(8):
                        nc.tensor.matmul(yp, lhsT=xT[:, kc, :],
                                         rhs=fcw[:, kc, c],
                                         start=False, stop=(kc == 7))
                    nc.scalar.activation(out=y_bf[:, c], in_=yp, func=AF.Silu)
                # x2 = conv*y + x   (into xt)
                nc.vector.tensor_mul(out=cv2, in0=cv1, in1=y_bf)
                nc.vector.tensor_add(out=xt, in0=cv2, in1=xt)
                # LN2 -> x3 (bf16)
                x3_bf = wbf.tile([128, D], BF16, tag="x3_bf")
                layer_norm(xt, x3_bf)
                x3T = transpose_bf(x3_bf, 8, "x3T")
                # FFN1 + silu
                h1 = wbf.tile([128, FF], BF16, tag="h1")
                for nf in range(4):
                    c = slice(nf * 512, (nf + 1) * 512)
                    hp = ppg.tile([128, 512], F32, tag="gemm")
                    nc.tensor.matmul(hp, lhsT=ones_bf, rhs=b1r[:, c],
                                     start=True, stop=False)
                    for kc in range(8):
                        nc.tensor.matmul(hp, lhsT=x3T[:, kc, :],
                                         rhs=w1[:, kc, c],
                                         start=False, stop=(kc == 7))
                    nc.scalar.activation(out=h1[:, c], in_=hp, func=AF.Silu)
                h1T = transpose_bf(h1, 16, "h1T")
                # FFN2; out = x2 + h  (into xt, then DMA out)
                for nd in range(2):
                    c = slice(nd * 512, (nd + 1) * 512)
                    op_ = ppg.tile([128, 512], F32, tag="gemm")
                    nc.tensor.matmul(op_, lhsT=ones_bf, rhs=b2r[:, c],
                                     start=True, stop=False)
                    for kc in range(16):
                        nc.tensor.matmul(op_, lhsT=h1T[:, kc, :],
                                         rhs=w2[:, kc, c],
                                         start=False, stop=(kc == 15))
                    nc.vector.tensor_add(out=xt[:, c], in0=op_, in1=xt[:, c])
                nc.sync.dma_start(out=out_p[i * 128:(i + 1) * 128, :], in_=xt)

            wc_cm.__exit__(None, None, None)

    nc.compile()
    return nc


def _tables(ln_g, ph_re, ph_im, phi_re, phi_im):
    ph = ph_re.astype(np.complex128) + 1j * ph_im.astype(np.complex128)
    phi = phi_re.astype(np.complex128) + 1j * phi_im.astype(np.complex128)
    aa = np.abs(ph)
    th = np.angle(ph)
    lg = -aa + 1j * th                    # log(phz)
    g = ln_g.astype(np.complex128)

    def pw(t):
        t = np.asarray(t, np.float64)
        return np.exp(t[..., None] * lg[None, :])

    def dm(v):          # [D] -> channel-major [128, NDT]
        return np.ascontiguousarray(v.reshape(NDT, 128).T)

    j16 = (np.arange(128) % 16).astype(np.float64)
    p2 = g[None, :] * phi[None, :] * pw(-j16)            # [128, D]
    a_t = pw(j16)                                        # [128, D]
    t16 = pw(16.0 * np.arange(128) + 1.0)                # [128, D]

    p16_d = pw(np.float64(16))[0]
    p15_d = pw(np.float64(15))[0]
    p2048_d = pw(np.float64(R))[0]
    # rotation tables: rot[b, d] = exp(-16i*theta_d*b) = rotc - i*rots
    bb = np.arange(NBLK, dtype=np.float64)
    rotc_bd = np.cos(16.0 * th[None, :] * bb[:, None])   # [NBLK, D]
    rots_bd = np.sin(16.0 * th[None, :] * bb[:, None])

    def dm3(v_bd):      # [NBLK, D] -> [128, NDT, NBLK]
        return np.ascontiguousarray(
            v_bd.reshape(NBLK, NDT, 128).transpose(2, 1, 0))

    r16_d = np.exp(-16.0 * aa)                           # [D] real decay

    bdt = np.zeros((128, 128), np.float32)
    for j in range(128):
        for ii in range(j, (j // 16) * 16 + 16):
            bdt[j, ii] = 1.0
    e8 = np.zeros((128, 16 * 128), np.float32)
    for col in range(16 * 128):
        e8[col // 16, col] = 1.0
    ws = np.zeros((128, 16 * 128), np.float32)
    for i in range(16):
        for j in range(128):
            ws[j, i * 128 + 8 * i + j // 16] = 1.0

    f = np.float32
    return dict(
        p2_re=f(p2.real), p2_im=f(p2.imag),
        a_re=f(a_t.real), a_im=f(a_t.imag),
        t16_re=f(t16.real), t16_im=f(t16.imag),
        bdt=bdt, e8=e8, ws=ws,
        rotc=f(dm3(rotc_bd)), rots=f(dm3(rots_bd)),
        r16=f(dm(r16_d)),
        p16_re=f(dm(p16_d.real)), p16_im=f(dm(p16_d.imag)),
        p15_re=f(dm(p15_d.real)), p15_im=f(dm(p15_d.imag)),
    ), p2048_d


def kernel(x, ln_g, ln_b, fc_w, fc_b, w1, b1, w2, b2,
           ph_re, ph_im, phi_re, phi_im, lci_re, lci_im, _trace=False,
           **_tr_kw):
    x = np.asarray(x, np.float32)
    tabs, p2048_d = _tables(np.asarray(ln_g, np.float32),
                            np.asarray(ph_re), np.asarray(ph_im),
                            np.asarray(phi_re), np.asarray(phi_im))
    lci = np.asarray(lci_re).astype(np.complex128) \
        + 1j * np.asarray(lci_im).astype(np.complex128)

    ln_g_f = np.asarray(ln_g, np.float32)
    w1_eff = (ln_g_f[:, None] * np.asarray(w1, np.float32))
    b1_eff = np.asarray(b1, np.float32) + \
        np.asarray(ln_b, np.float32) @ np.asarray(w1, np.float32)

    def dm(v):
        return np.ascontiguousarray(v.reshape(NDT, 128).T)

    f = np.float32
    common = dict(tabs)
    common["fcw"] = np.ascontiguousarray(
        np.asarray(fc_w, np.float32).reshape(8, 128, D)).astype(BF)
    common["w1"] = np.ascontiguousarray(w1_eff.reshape(8, 128, FF)).astype(BF)
    common["w2"] = np.ascontiguousarray(
        np.asarray(w2, np.float32).reshape(16, 128, D)).astype(BF)
    common["fcb"] = np.asarray(fc_b, np.float32).reshape(1, D).astype(BF)
    common["b1r"] = b1_eff.reshape(1, FF).astype(BF)
    common["b2r"] = np.asarray(b2, np.float32).reshape(1, D).astype(BF)

    in_maps = []
    for c in range(NCORES):
        b, h = c // 2, c % 2
        m = dict(common)
        m["x"] = np.ascontiguousarray(x[b, h * R:(h + 1) * R, :])
        sel = np.zeros((8, 1), np.float32)
        if h == 1:
            sel[c - 1, 0] = 1.0
        m["sel8"] = sel
        if h == 0:
            lcie = lci
            xb = p2048_d * lci
        else:
            lcie = np.zeros(D, np.complex128)
            xb = np.zeros(D, np.complex128)
        m["lcie_re"] = f(lcie.real).reshape(1, D)
        m["lcie_im"] = f(lcie.imag).reshape(1, D)
        m["xb_re"] = f(dm(xb.real))
        m["xb_im"] = f(dm(xb.imag))
        in_maps.append(m)

    if "nc" not in _CACHE:
        _CACHE["nc"] = build_nc()
    nc = _CACHE["nc"]
    res = run_bass_kernel_spmd(nc, in_maps, core_ids=list(range(NCORES)),
                               trace=_trace, **_tr_kw)
    out = np.empty((B, L, D), np.float32)
    for c in range(NCORES):
        b, h = c // 2, c % 2
        out[b, h * R:(h + 1) * R, :] = res.results[c]["out"]
    if _trace:
        return out, res
    return out
